# revision 8
# baseline (speedup 1.0000x reference)
"""MoE (dense routing variant) Trainium2 Bass kernel.

Contract: kernel(**inputs) takes FULL numpy inputs (as produced by
setup_inputs()) and returns the FULL [B, S, D] float32 output.

Strategy: data-parallel over tokens. Each of the 8 NeuronCores gets
N/8 = 1024 tokens and runs the full gate + all 8 experts for its tokens,
so no cross-core communication is needed. All big matmuls run in bf16
(1 cycle/row on the PE) with fp32 PSUM accumulation.

Per-core layout (token index always on the free axis):
  xt  [128, DC*NT]  bf16   x^T by d-chunk: xt[k, dc*NT+n] = x[n, dc*128+k]
  w1  [E, 128, HC*DC*128]  bf16  lhsT tiles: w1[e][k, (hh*DC+dc)*128+m] = w1[e, dc*128+k, hh*128+m]
  w2  [E, 128, DC*HC*128]  bf16  lhsT tiles: w2[e][k, (dd*HC+hh)*128+m] = w2[e, hh*128+k, dd*128+m]
  gwt [128, DC*E]   bf16   gate_w^T tiles: gwt[k, dc*E+m] = gate_w[m, dc*128+k]
  yt  [128, DC*NT]  f32    y^T output: yt[k, dd*NT+n] = y[n, dd*128+k]
"""

import os

import ml_dtypes
import numpy as np

import concourse.bass as bass
import concourse.tile as tile
from concourse import bacc, mybir
from concourse.bass_utils import run_bass_kernel_spmd

# Problem dims (hardcoded per spec).
B, S, D, H, E = 4, 2048, 1024, 2048, 8
N = B * S
NCORES = 8
NT = N // NCORES          # tokens per core
P = 128
DC = D // P               # 8 d-chunks
HC = H // P               # 16 h-chunks
MMN = 512                 # matmul moving free dim (one PSUM bank of fp32)
NTT = NT // MMN           # 2 token tiles for matmuls
TKT = NT // P             # 8 token tiles of 128 for the gate

F32 = mybir.dt.float32
BF16 = mybir.dt.bfloat16
AF = mybir.ActivationFunctionType
AX = mybir.AxisListType
EPS = float(np.finfo(np.float32).eps)
INV_TEMP = 2.0            # 1 / TEMPERATURE


def _bcast_rows(handle_row_ap: bass.AP, nrows: int) -> bass.AP:
    """Broadcast a [cols]-shaped DRAM AP across `nrows` partitions."""
    return bass.AP(
        tensor=handle_row_ap.tensor,
        offset=handle_row_ap.offset,
        ap=[[0, nrows]] + handle_row_ap.ap,
    )


def build_nc() -> bass.Bass:
    # Bacc (not raw Bass): its compile pass splits multi-sem waits into
    # event semaphores — TRN2 instructions carry at most one wait.
    nc = bacc.Bacc()

    xt_d = nc.declare_dram_parameter("xt", [P, DC * NT], BF16, isOutput=False)
    w1_d = nc.declare_dram_parameter("w1", [E, P, HC * DC * P], BF16, isOutput=False)
    w2_d = nc.declare_dram_parameter("w2", [E, P, DC * HC * P], BF16, isOutput=False)
    gwt_d = nc.declare_dram_parameter("gwt", [P, DC * E], BF16, isOutput=False)
    gb_d = nc.declare_dram_parameter("gb", [1, E], F32, isOutput=False)
    rw_d = nc.declare_dram_parameter("rw", [1, E], F32, isOutput=False)
    b1_d = nc.declare_dram_parameter("b1", [E, P, HC], F32, isOutput=False)
    b2s_d = nc.declare_dram_parameter("b2s", [P, DC], F32, isOutput=False)
    yt_d = nc.declare_dram_parameter("yt", [P, DC * NT], F32, isOutput=True)

    with tile.TileContext(nc) as tc:
        with (
            tc.tile_pool(name="const", bufs=1) as const,
            tc.tile_pool(name="xp", bufs=1) as xp,
            tc.tile_pool(name="w1p", bufs=1) as w1p,
            tc.tile_pool(name="w2p", bufs=1) as w2p,
            tc.tile_pool(name="b1p", bufs=2) as b1p,
            tc.tile_pool(name="ghp", bufs=20) as ghp,
            tc.tile_pool(name="gatep", bufs=4) as gatep,
            tc.tile_pool(name="dramp", bufs=1, space="DRAM") as dramp,
            tc.tile_pool(name="ps1", bufs=3, space="PSUM") as ps1,
            tc.tile_pool(name="ps2", bufs=3, space="PSUM") as ps2,
            tc.tile_pool(name="psg", bufs=2, space="PSUM") as psg,
        ):
            # ---------- loads ----------
            xts = xp.tile([P, DC * NT], BF16)
            nc.sync.dma_start(out=xts, in_=xt_d[:, :])

            gwts = const.tile([P, DC * E], BF16)
            nc.gpsimd.dma_start(out=gwts, in_=gwt_d[:, :])
            gb_bc = const.tile([P, E], F32)
            nc.gpsimd.dma_start(out=gb_bc, in_=_bcast_rows(gb_d[0, :], P))
            rw_bc = const.tile([P, E], F32)
            nc.gpsimd.dma_start(out=rw_bc, in_=_bcast_rows(rw_d[0, :], P))
            b2t = const.tile([P, DC], F32)
            nc.gpsimd.dma_start(out=b2t, in_=b2s_d[:, :])

            # ---------- gate (token-major [128, E] tiles) ----------
            eps_t = const.tile([P, 1], F32)
            nc.vector.memset(eps_t, EPS)
            zz = const.tile([P, 1], F32)
            nc.vector.memset(zz, 0.0)
            ngcol = const.tile([P, TKT], F32)
            for t in range(TKT):
                pg = psg.tile([P, E], F32)
                for dc in range(DC):
                    nc.tensor.matmul(
                        pg,
                        xts[:, dc * NT + t * P : dc * NT + (t + 1) * P],
                        gwts[:, dc * E : (dc + 1) * E],
                        start=(dc == 0),
                        stop=(dc == DC - 1),
                    )
                lg = gatep.tile([P, E], F32)
                nc.vector.tensor_add(lg, pg, gb_bc)
                sq = gatep.tile([P, E], F32)
                nc.vector.tensor_mul(sq, lg, lg)
                ms = gatep.tile([P, 1], F32)
                nc.vector.reduce_sum(ms, sq, axis=AX.X)
                # sqrt(mean + eps); eps matches torch RMSNorm(eps=None) -> f32 eps
                srt = gatep.tile([P, 1], F32)
                nc.scalar.activation(srt, ms, AF.Sqrt, bias=eps_t, scale=1.0 / E)
                inv = gatep.tile([P, 1], F32)
                nc.vector.reciprocal(inv, srt)
                ln = gatep.tile([P, E], F32)
                nc.vector.tensor_scalar_mul(ln, lg, inv)
                ln2 = gatep.tile([P, E], F32)
                nc.vector.tensor_mul(ln2, ln, rw_bc)
                ex = gatep.tile([P, E], F32)
                nc.scalar.activation(ex, ln2, AF.Exp, bias=zz, scale=INV_TEMP)
                den = gatep.tile([P, 1], F32)
                nc.vector.reduce_sum(den, ex, axis=AX.X)
                gm = gatep.tile([P, 1], F32)
                nc.vector.reduce_max(gm, ex, axis=AX.X)
                rd = gatep.tile([P, 1], F32)
                nc.vector.reciprocal(rd, den)
                gg = gatep.tile([P, 1], F32)
                nc.vector.tensor_mul(gg, gm, rd)
                uu = gatep.tile([P, 1], F32)
                nc.vector.tensor_scalar_add(uu, gg, 1e-6)
                ru = gatep.tile([P, 1], F32)
                nc.vector.reciprocal(ru, uu)
                nc.vector.tensor_mul(ngcol[:, t : t + 1], gg, ru)

            # norm_g: [128 tok, TKT tile] -> DRAM token-major -> broadcast row
            ngd = dramp.tile([NT], F32)
            nc.gpsimd.dma_start(out=ngd.rearrange("(a b) -> b a", b=P), in_=ngcol)
            ngb = const.tile([P, NT], F32)
            nc.gpsimd.dma_start(out=ngb, in_=_bcast_rows(ngd[:], P))

            # ---------- experts ----------
            y_acc = []
            for dd in range(DC):
                ya = const.tile([P, NT], F32, tag=f"yacc{dd}", name=f"yacc{dd}")
                y_acc.append(ya)

            for e in range(E):
                w1t = w1p.tile([P, HC * DC * P], BF16, tag="w1t", name=f"w1t{e}")
                nc.sync.dma_start(out=w1t, in_=w1_d[e])
                w2t = w2p.tile([P, DC * HC * P], BF16, tag="w2t", name=f"w2t{e}")
                nc.sync.dma_start(out=w2t, in_=w2_d[e])
                b1t = b1p.tile([P, HC], F32, tag="b1t", name=f"b1t{e}")
                nc.gpsimd.dma_start(out=b1t, in_=b1_d[e])

                # layer 1: gh[hh] = gelu(w1[e]^T x^T + b1), bf16 [128, NT]
                ghs = []
                for hh in range(HC):
                    gh = ghp.tile([P, NT], BF16, tag="gh", name=f"gh{e}_{hh}")
                    for t in range(NTT):
                        p1 = ps1.tile([P, MMN], F32)
                        for dc in range(DC):
                            nc.tensor.matmul(
                                p1,
                                w1t[:, (hh * DC + dc) * P : (hh * DC + dc + 1) * P],
                                xts[:, dc * NT + t * MMN : dc * NT + (t + 1) * MMN],
                                start=(dc == 0),
                                stop=(dc == DC - 1),
                            )
                        nc.scalar.activation(
                            gh[:, t * MMN : (t + 1) * MMN],
                            p1,
                            AF.Gelu,
                            bias=b1t[:, hh : hh + 1],
                        )
                    ghs.append(gh)

                # layer 2: y_acc[dd] += w2[e]^T gh  (+ sum_e b2 on e==0)
                for dd in range(DC):
                    for t in range(NTT):
                        p2 = ps2.tile([P, MMN], F32)
                        for hh in range(HC):
                            nc.tensor.matmul(
                                p2,
                                w2t[:, (dd * HC + hh) * P : (dd * HC + hh + 1) * P],
                                ghs[hh][:, t * MMN : (t + 1) * MMN],
                                start=(hh == 0),
                                stop=(hh == HC - 1),
                            )
                        ysl = y_acc[dd][:, t * MMN : (t + 1) * MMN]
                        if e == 0:
                            nc.vector.tensor_scalar_add(ysl, p2, b2t[:, dd : dd + 1])
                        else:
                            nc.vector.tensor_add(ysl, ysl, p2)

            # ---------- combine + store ----------
            for dd in range(DC):
                nc.vector.tensor_mul(y_acc[dd], y_acc[dd], ngb)
                nc.sync.dma_start(out=yt_d[:, dd * NT : (dd + 1) * NT], in_=y_acc[dd])

    nc.finalize()
    return nc


_CACHE: dict = {}
LAST_RESULTS = None


def _get_nc() -> bass.Bass:
    if "nc" not in _CACHE:
        _CACHE["nc"] = build_nc()
    return _CACHE["nc"]


def _prep_shared(gate_w, gate_b, rms_w, w1, b1, w2, b2):
    bf16 = ml_dtypes.bfloat16
    f32 = np.float32
    w1h = (
        np.asarray(w1, f32)
        .reshape(E, DC, P, HC, P)
        .transpose(0, 2, 3, 1, 4)
        .astype(bf16)
        .reshape(E, P, HC * DC * P)
    )
    w2h = (
        np.asarray(w2, f32)
        .reshape(E, HC, P, DC, P)
        .transpose(0, 2, 3, 1, 4)
        .astype(bf16)
        .reshape(E, P, DC * HC * P)
    )
    gwth = (
        np.asarray(gate_w, f32)
        .reshape(E, DC, P)
        .transpose(2, 1, 0)
        .astype(bf16)
        .reshape(P, DC * E)
    )
    gbh = np.asarray(gate_b, f32).reshape(1, E)
    rwh = np.asarray(rms_w, f32).reshape(1, E)
    b1h = np.ascontiguousarray(
        np.asarray(b1, f32).reshape(E, HC, P).transpose(0, 2, 1)
    )
    b2sh = np.ascontiguousarray(np.asarray(b2, f32).sum(axis=0).reshape(DC, P).T)
    return {
        "w1": w1h,
        "w2": w2h,
        "gwt": gwth,
        "gb": gbh,
        "rw": rwh,
        "b1": b1h,
        "b2s": b2sh,
    }


def make_in_maps(x, gate_w, gate_b, rms_w, w1, b1, w2, b2):
    bf16 = ml_dtypes.bfloat16
    shared = _prep_shared(gate_w, gate_b, rms_w, w1, b1, w2, b2)
    xf = np.asarray(x, np.float32).reshape(N, D)
    in_maps = []
    for c in range(NCORES):
        xc = xf[c * NT : (c + 1) * NT]
        xth = (
            xc.reshape(NT, DC, P).transpose(2, 1, 0).astype(bf16).reshape(P, DC * NT)
        )
        m = {"xt": xth}
        m.update(shared)
        in_maps.append(m)
    return in_maps


def gather_out(results) -> np.ndarray:
    outs = []
    for c in range(NCORES):
        ytc = np.asarray(results[c]["yt"], np.float32)
        yc = ytc.reshape(P, DC, NT).transpose(2, 1, 0).reshape(NT, D)
        outs.append(yc)
    return np.concatenate(outs, axis=0).reshape(B, S, D)


def kernel(x, gate_w, gate_b, rms_w, w1, b1, w2, b2) -> np.ndarray:
    global LAST_RESULTS
    in_maps = make_in_maps(x, gate_w, gate_b, rms_w, w1, b1, w2, b2)
    nc = _get_nc()
    res = run_bass_kernel_spmd(
        nc,
        in_maps,
        list(range(NCORES)),
        trace=bool(os.environ.get("MOE_TRACE")),
    )
    LAST_RESULTS = res
    return gather_out(res.results)


# revision 10
# speedup vs baseline: 1.0102x; 1.0102x over previous
"""MoE (dense routing variant) Trainium2 Bass kernel.

Contract: kernel(**inputs) takes FULL numpy inputs (as produced by
setup_inputs()) and returns the FULL [B, S, D] float32 output.

Strategy: data-parallel over tokens. Each of the 8 NeuronCores gets
N/8 = 1024 tokens and runs the full gate + all 8 experts for its tokens,
so no cross-core communication is needed. All big matmuls run in bf16
(1 cycle/row on the PE) with fp32 PSUM accumulation.

Per-core layout (token index always on the free axis):
  xt  [128, DC*NT]  bf16   x^T by d-chunk: xt[k, dc*NT+n] = x[n, dc*128+k]
  w1  [E, 128, HC*DC*128]  bf16  lhsT tiles: w1[e][k, (hh*DC+dc)*128+m] = w1[e, dc*128+k, hh*128+m]
  w2  [E, 128, DC*HC*128]  bf16  lhsT tiles: w2[e][k, (dd*HC+hh)*128+m] = w2[e, hh*128+k, dd*128+m]
  gwt [128, DC*E]   bf16   gate_w^T tiles: gwt[k, dc*E+m] = gate_w[m, dc*128+k]
  yt  [128, DC*NT]  f32    y^T output: yt[k, dd*NT+n] = y[n, dd*128+k]
"""

import os

import ml_dtypes
import numpy as np

import concourse.bass as bass
import concourse.tile as tile
from concourse import bacc, mybir
from concourse.bass_utils import run_bass_kernel_spmd

# Problem dims (hardcoded per spec).
B, S, D, H, E = 4, 2048, 1024, 2048, 8
N = B * S
NCORES = 8
NT = N // NCORES          # tokens per core
P = 128
DC = D // P               # 8 d-chunks
HC = H // P               # 16 h-chunks
MMN = 512                 # matmul moving free dim (one PSUM bank of fp32)
NTT = NT // MMN           # 2 token tiles for matmuls
TKT = NT // P             # 8 token tiles of 128 for the gate

F32 = mybir.dt.float32
BF16 = mybir.dt.bfloat16
AF = mybir.ActivationFunctionType
AX = mybir.AxisListType
EPS = float(np.finfo(np.float32).eps)
INV_TEMP = 2.0            # 1 / TEMPERATURE


def _bcast_rows(handle_row_ap: bass.AP, nrows: int) -> bass.AP:
    """Broadcast a [cols]-shaped DRAM AP across `nrows` partitions."""
    return bass.AP(
        tensor=handle_row_ap.tensor,
        offset=handle_row_ap.offset,
        ap=[[0, nrows]] + handle_row_ap.ap,
    )


def build_nc() -> bass.Bass:
    # Bacc (not raw Bass): its compile pass splits multi-sem waits into
    # event semaphores — TRN2 instructions carry at most one wait.
    nc = bacc.Bacc()

    xt_d = nc.declare_dram_parameter("xt", [P, DC * NT], BF16, isOutput=False)
    w1_d = nc.declare_dram_parameter("w1", [E, P, HC * DC * P], BF16, isOutput=False)
    w2_d = nc.declare_dram_parameter("w2", [E, P, DC * HC * P], BF16, isOutput=False)
    gwt_d = nc.declare_dram_parameter("gwt", [P, DC * E], BF16, isOutput=False)
    gb_d = nc.declare_dram_parameter("gb", [1, E], F32, isOutput=False)
    rw_d = nc.declare_dram_parameter("rw", [1, E], F32, isOutput=False)
    b1_d = nc.declare_dram_parameter("b1", [E, P, HC], F32, isOutput=False)
    b2s_d = nc.declare_dram_parameter("b2s", [P, DC], F32, isOutput=False)
    yt_d = nc.declare_dram_parameter("yt", [P, DC * NT], F32, isOutput=True)

    with tile.TileContext(nc) as tc:
        with (
            tc.tile_pool(name="const", bufs=1) as const,
            tc.tile_pool(name="xp", bufs=1) as xp,
            tc.tile_pool(name="w1p", bufs=1) as w1p,
            tc.tile_pool(name="w2p", bufs=1) as w2p,
            tc.tile_pool(name="b1p", bufs=2) as b1p,
            tc.tile_pool(name="ghp", bufs=20) as ghp,
            tc.tile_pool(name="gatep", bufs=4) as gatep,
            tc.tile_pool(name="dramp", bufs=1, space="DRAM") as dramp,
            tc.tile_pool(name="ps1", bufs=3, space="PSUM") as ps1,
            tc.tile_pool(name="ps2", bufs=3, space="PSUM") as ps2,
            tc.tile_pool(name="psg", bufs=2, space="PSUM") as psg,
        ):
            # ---------- loads ----------
            xts = xp.tile([P, DC * NT], BF16)
            nc.sync.dma_start(out=xts, in_=xt_d[:, :])

            gwts = const.tile([P, DC * E], BF16)
            nc.gpsimd.dma_start(out=gwts, in_=gwt_d[:, :])
            gb_bc = const.tile([P, E], F32)
            nc.gpsimd.dma_start(out=gb_bc, in_=_bcast_rows(gb_d[0, :], P))
            rw_bc = const.tile([P, E], F32)
            nc.gpsimd.dma_start(out=rw_bc, in_=_bcast_rows(rw_d[0, :], P))
            b2t = const.tile([P, DC], F32)
            nc.gpsimd.dma_start(out=b2t, in_=b2s_d[:, :])

            # ---------- gate (token-major [128, E] tiles) ----------
            eps_t = const.tile([P, 1], F32)
            nc.vector.memset(eps_t, EPS)
            zz = const.tile([P, 1], F32)
            nc.vector.memset(zz, 0.0)
            ngcol = const.tile([P, TKT], F32)
            for t in range(TKT):
                pg = psg.tile([P, E], F32)
                for dc in range(DC):
                    nc.tensor.matmul(
                        pg,
                        xts[:, dc * NT + t * P : dc * NT + (t + 1) * P],
                        gwts[:, dc * E : (dc + 1) * E],
                        start=(dc == 0),
                        stop=(dc == DC - 1),
                    )
                lg = gatep.tile([P, E], F32)
                nc.vector.tensor_add(lg, pg, gb_bc)
                sq = gatep.tile([P, E], F32)
                nc.vector.tensor_mul(sq, lg, lg)
                ms = gatep.tile([P, 1], F32)
                nc.vector.reduce_sum(ms, sq, axis=AX.X)
                # sqrt(mean + eps); eps matches torch RMSNorm(eps=None) -> f32 eps
                srt = gatep.tile([P, 1], F32)
                nc.scalar.activation(srt, ms, AF.Sqrt, bias=eps_t, scale=1.0 / E)
                inv = gatep.tile([P, 1], F32)
                nc.vector.reciprocal(inv, srt)
                ln = gatep.tile([P, E], F32)
                nc.vector.tensor_scalar_mul(ln, lg, inv)
                ln2 = gatep.tile([P, E], F32)
                nc.vector.tensor_mul(ln2, ln, rw_bc)
                ex = gatep.tile([P, E], F32)
                nc.scalar.activation(ex, ln2, AF.Exp, bias=zz, scale=INV_TEMP)
                den = gatep.tile([P, 1], F32)
                nc.vector.reduce_sum(den, ex, axis=AX.X)
                gm = gatep.tile([P, 1], F32)
                nc.vector.reduce_max(gm, ex, axis=AX.X)
                rd = gatep.tile([P, 1], F32)
                nc.vector.reciprocal(rd, den)
                gg = gatep.tile([P, 1], F32)
                nc.vector.tensor_mul(gg, gm, rd)
                uu = gatep.tile([P, 1], F32)
                nc.vector.tensor_scalar_add(uu, gg, 1e-6)
                ru = gatep.tile([P, 1], F32)
                nc.vector.reciprocal(ru, uu)
                nc.vector.tensor_mul(ngcol[:, t : t + 1], gg, ru)

            # norm_g: [128 tok, TKT tile] -> DRAM token-major -> broadcast row
            ngd = dramp.tile([NT], F32)
            nc.gpsimd.dma_start(out=ngd.rearrange("(a b) -> b a", b=P), in_=ngcol)
            ngb = const.tile([P, NT], F32)
            nc.gpsimd.dma_start(out=ngb, in_=_bcast_rows(ngd[:], P))

            # ---------- experts ----------
            y_acc = []
            for dd in range(DC):
                ya = const.tile([P, NT], F32, tag=f"yacc{dd}", name=f"yacc{dd}")
                y_acc.append(ya)

            for e in range(E):
                # per-hh-chunk DMAs so layer-1 hh=0 can start after 256KB,
                # not after the whole 4MB weight load
                w1t = w1p.tile([P, HC * DC * P], BF16, tag="w1t", name=f"w1t{e}")
                for hh in range(HC):
                    nc.sync.dma_start(
                        out=w1t[:, hh * DC * P : (hh + 1) * DC * P],
                        in_=w1_d[e][:, hh * DC * P : (hh + 1) * DC * P],
                    )
                b1t = b1p.tile([P, HC], F32, tag="b1t", name=f"b1t{e}")
                nc.gpsimd.dma_start(out=b1t, in_=b1_d[e])

                # layer 1: gh[hh] = gelu(w1[e]^T x^T + b1), bf16 [128, NT]
                ghs = []
                for hh in range(HC):
                    gh = ghp.tile([P, NT], BF16, tag="gh", name=f"gh{e}_{hh}")
                    for t in range(NTT):
                        p1 = ps1.tile([P, MMN], F32)
                        for dc in range(DC):
                            nc.tensor.matmul(
                                p1,
                                w1t[:, (hh * DC + dc) * P : (hh * DC + dc + 1) * P],
                                xts[:, dc * NT + t * MMN : dc * NT + (t + 1) * MMN],
                                start=(dc == 0),
                                stop=(dc == DC - 1),
                            )
                        nc.scalar.activation(
                            gh[:, t * MMN : (t + 1) * MMN],
                            p1,
                            AF.Gelu,
                            bias=b1t[:, hh : hh + 1],
                        )
                    ghs.append(gh)

                # w2 prefetch after layer-1 emission (lower priority than the
                # weights layer 1 is stalled on), per-dd chunks
                w2t = w2p.tile([P, DC * HC * P], BF16, tag="w2t", name=f"w2t{e}")
                for dd in range(DC):
                    nc.sync.dma_start(
                        out=w2t[:, dd * HC * P : (dd + 1) * HC * P],
                        in_=w2_d[e][:, dd * HC * P : (dd + 1) * HC * P],
                    )

                # layer 2: y_acc[dd] += w2[e]^T gh  (+ sum_e b2 on e==0);
                # on the last expert, fuse the norm_g combine + store so the
                # output DMAs overlap the tail of compute
                for dd in range(DC):
                    for t in range(NTT):
                        p2 = ps2.tile([P, MMN], F32)
                        for hh in range(HC):
                            nc.tensor.matmul(
                                p2,
                                w2t[:, (dd * HC + hh) * P : (dd * HC + hh + 1) * P],
                                ghs[hh][:, t * MMN : (t + 1) * MMN],
                                start=(hh == 0),
                                stop=(hh == HC - 1),
                            )
                        ysl = y_acc[dd][:, t * MMN : (t + 1) * MMN]
                        if e == 0:
                            nc.vector.tensor_scalar_add(ysl, p2, b2t[:, dd : dd + 1])
                        elif e < E - 1:
                            nc.vector.tensor_add(ysl, ysl, p2)
                        else:
                            nc.vector.tensor_add(ysl, ysl, p2)
                            nc.vector.tensor_mul(
                                ysl, ysl, ngb[:, t * MMN : (t + 1) * MMN]
                            )
                            nc.sync.dma_start(
                                out=yt_d[:, dd * NT + t * MMN : dd * NT + (t + 1) * MMN],
                                in_=ysl,
                            )

    nc.finalize()
    return nc


_CACHE: dict = {}
LAST_RESULTS = None


def _get_nc() -> bass.Bass:
    if "nc" not in _CACHE:
        _CACHE["nc"] = build_nc()
    return _CACHE["nc"]


def _prep_shared(gate_w, gate_b, rms_w, w1, b1, w2, b2):
    bf16 = ml_dtypes.bfloat16
    f32 = np.float32
    w1h = (
        np.asarray(w1, f32)
        .reshape(E, DC, P, HC, P)
        .transpose(0, 2, 3, 1, 4)
        .astype(bf16)
        .reshape(E, P, HC * DC * P)
    )
    w2h = (
        np.asarray(w2, f32)
        .reshape(E, HC, P, DC, P)
        .transpose(0, 2, 3, 1, 4)
        .astype(bf16)
        .reshape(E, P, DC * HC * P)
    )
    gwth = (
        np.asarray(gate_w, f32)
        .reshape(E, DC, P)
        .transpose(2, 1, 0)
        .astype(bf16)
        .reshape(P, DC * E)
    )
    gbh = np.asarray(gate_b, f32).reshape(1, E)
    rwh = np.asarray(rms_w, f32).reshape(1, E)
    b1h = np.ascontiguousarray(
        np.asarray(b1, f32).reshape(E, HC, P).transpose(0, 2, 1)
    )
    b2sh = np.ascontiguousarray(np.asarray(b2, f32).sum(axis=0).reshape(DC, P).T)
    return {
        "w1": w1h,
        "w2": w2h,
        "gwt": gwth,
        "gb": gbh,
        "rw": rwh,
        "b1": b1h,
        "b2s": b2sh,
    }


def make_in_maps(x, gate_w, gate_b, rms_w, w1, b1, w2, b2):
    bf16 = ml_dtypes.bfloat16
    shared = _prep_shared(gate_w, gate_b, rms_w, w1, b1, w2, b2)
    xf = np.asarray(x, np.float32).reshape(N, D)
    in_maps = []
    for c in range(NCORES):
        xc = xf[c * NT : (c + 1) * NT]
        xth = (
            xc.reshape(NT, DC, P).transpose(2, 1, 0).astype(bf16).reshape(P, DC * NT)
        )
        m = {"xt": xth}
        m.update(shared)
        in_maps.append(m)
    return in_maps


def gather_out(results) -> np.ndarray:
    outs = []
    for c in range(NCORES):
        ytc = np.asarray(results[c]["yt"], np.float32)
        yc = ytc.reshape(P, DC, NT).transpose(2, 1, 0).reshape(NT, D)
        outs.append(yc)
    return np.concatenate(outs, axis=0).reshape(B, S, D)


def kernel(x, gate_w, gate_b, rms_w, w1, b1, w2, b2) -> np.ndarray:
    global LAST_RESULTS
    in_maps = make_in_maps(x, gate_w, gate_b, rms_w, w1, b1, w2, b2)
    nc = _get_nc()
    res = run_bass_kernel_spmd(
        nc,
        in_maps,
        list(range(NCORES)),
        trace=bool(os.environ.get("MOE_TRACE")),
    )
    LAST_RESULTS = res
    return gather_out(res.results)
